# revision 2
# baseline (speedup 1.0000x reference)
"""Local (windowed) attention with rotary embeddings on 8 TRN2 NeuronCores. v2.

Same math as the baseline kernel (see kernel.py docstring), rebuilt around the
CoreSim cost model:
  - everything bf16 (tolerance 2e-2; bf16 matmul/transpose = 1 cyc/col vs
    fp32's 4/2; DVE gets 2x on bf16; DMA volume halves).
  - QK for odd windows uses operands at partition base 64 directly (validated
    in sim) -> the GPSIMD partition-shift copy is gone.
  - causal mask = multiply by a precomputed 0/1 bf16 mask tile on DVE
    (strided view over both own-chunk halves) instead of GPSIMD affine_select.
  - softmax denominator via 1-column ones matmuls accumulated into the PV
    PSUM tile (no vo_row copy), normalize = tensor divide on GPSIMD with a
    stride-0 broadcast divisor (no reciprocal + no ACT copies).
  - rotary products: qA/qB cos (and sin) fused into single DVE ops via a
    stride-0 double-read of q_row; k rotation on GPSIMD.
  - exp on ACT batched over 2 window-pairs ([128, 1024]).
"""

import numpy as np
import ml_dtypes

import concourse.bass as bass
import concourse.bacc as bacc
import concourse.tile as tile
from concourse import mybir
from concourse.bass_utils import run_bass_kernel_spmd

B, H, N, D = 4, 8, 4096, 64
WIN = 128
NW = N // WIN            # 32 windows per row
NCORES = 8
ROWS = B * H             # 32 packed batch rows
RPC = ROWS // NCORES     # 4 rows per core
ROPE = 10000.0
SCALE = D ** -0.5

F32 = mybir.dt.float32
BF16 = mybir.dt.bfloat16
BF = ml_dtypes.bfloat16


def _rot_consts():
    """Host-side rotary constant tables, [WIN, D] each, bf16."""
    inv = 1.0 / (ROPE ** (np.arange(0, D, 2, dtype=np.float64) / D))  # [D/2]

    def mats(t):
        fr = t[:, None] * inv[None, :]
        fr = np.concatenate([fr, fr], axis=-1)  # [WIN, D]
        return np.cos(fr), np.sin(fr)

    i = np.arange(WIN, dtype=np.float64)
    cosA, sinA = mats(i)          # q angle i     (vs own chunk, k angle jj')
    cosB, sinB = mats(i + WIN)    # q angle i+128 (vs prev chunk)
    cosK, sinK = mats(i)          # k angle jj'

    def fold_sin(s):
        f = s.copy()
        f[:, : D // 2] = -f[:, : D // 2]
        return f

    out = dict(
        cqA=cosA * SCALE, sqA=fold_sin(sinA) * SCALE,
        cqB=cosB * SCALE, sqB=fold_sin(sinB) * SCALE,
        cK=cosK, sK=fold_sin(sinK),
    )
    return {k: v.astype(BF) for k, v in out.items()}


CONST_SHAPES = {n: [WIN, D] for n in ("cqA", "sqA", "cqB", "sqB", "cK", "sK")}


def build_bass():
    nc = bacc.Bacc("TRN2", target_bir_lowering=False)
    q_d = nc.declare_dram_parameter("q", [RPC, WIN, NW, D], BF16, isOutput=False)
    k_d = nc.declare_dram_parameter("k", [RPC, WIN, NW, D], BF16, isOutput=False)
    v_d = nc.declare_dram_parameter("v", [RPC, WIN, NW, D], BF16, isOutput=False)
    consts_d = {
        name: nc.declare_dram_parameter(name, shape, BF16, isOutput=False)
        for name, shape in CONST_SHAPES.items()
    }
    mask_d = nc.declare_dram_parameter("maskT", [WIN, WIN], BF16, isOutput=False)
    o_d = nc.declare_dram_parameter("o", [RPC, WIN, NW, D], F32, isOutput=True)

    with tile.TileContext(nc) as tc:
        with (
            tc.tile_pool(name="singles", bufs=1) as singles,
            tc.tile_pool(name="rows", bufs=3) as rows,
            tc.tile_pool(name="rot", bufs=2) as rot,
            tc.tile_pool(name="sS", bufs=4) as s_pool,
            tc.tile_pool(name="exp", bufs=3) as exp_pool,
            tc.tile_pool(name="ptr", bufs=2, space="PSUM") as ptr_pool,
            tc.tile_pool(name="psim", bufs=2, space="PSUM") as psim_pool,
            tc.tile_pool(name="po", bufs=2, space="PSUM") as po_pool,
        ):
            # ---- row 0 input DMAs first so rotary can start ASAP, then
            # constants in first-use order.
            row_tiles = []
            for r in range(RPC):
                q_row = rows.tile([WIN, NW, D], BF16, tag="q_row")
                k_row = rows.tile([WIN, NW, D], BF16, tag="k_row")
                v_row = rows.tile([WIN, NW, D], BF16, tag="v_row")
                row_tiles.append((q_row, k_row, v_row))
            # first 8 windows of row-0 q/k land first so rotary starts ASAP
            nc.sync.dma_start(out=row_tiles[0][0][:, 0:8, :],
                              in_=q_d[0][:, 0:8, :])
            nc.sync.dma_start(out=row_tiles[0][1][:, 0:8, :],
                              in_=k_d[0][:, 0:8, :])
            c_sb = {}
            for name, shape in CONST_SHAPES.items():
                t = singles.tile(shape, BF16, tag=f"const_{name}")
                nc.sync.dma_start(out=t, in_=consts_d[name][:, :])
                c_sb[name] = t
            nc.sync.dma_start(out=row_tiles[0][0][:, 8:, :],
                              in_=q_d[0][:, 8:, :])
            nc.sync.dma_start(out=row_tiles[0][1][:, 8:, :],
                              in_=k_d[0][:, 8:, :])
            nc.sync.dma_start(out=row_tiles[0][2], in_=v_d[0])
            mask_sb = singles.tile([WIN, WIN], BF16, tag="maskT")
            nc.sync.dma_start(out=mask_sb, in_=mask_d[:, :])
            ones_sb = singles.tile([WIN, 1], BF16, tag="ones")
            nc.vector.memset(ones_sb, 1.0)

            # ---- identity built in-SBUF (no DMA wait): ones where col == p
            ident_sb = singles.tile([WIN, WIN], BF16, tag="ident")
            ones_bc = bass.AP(tensor=ones_sb.tensor, offset=ones_sb.offset,
                              ap=[list(ones_sb.ap[0]), [0, WIN]])
            nc.gpsimd.affine_select(
                out=ident_sb, in_=ones_bc,
                compare_op=mybir.AluOpType.is_equal,
                fill=0.0, base=0, pattern=[[1, WIN]], channel_multiplier=-1)

            # -30000 * identity: rhs of the causal-mask matmul
            negI_sb = singles.tile([WIN, WIN], BF16, tag="negI")
            nc.vector.tensor_scalar_mul(negI_sb, ident_sb, -30000.0)

            # ---- PE p-state warmup: ~3us of throwaway transposes during the
            # DMA-latency prologue so real work starts at full clock; dummy
            # exp pulls the ACT table load off the critical path.
            zwarm = singles.tile([WIN, 64], BF16, tag="zwarm")
            nc.vector.memset(zwarm, 0.0)
            wexp = singles.tile([WIN, 1], BF16, tag="wexp")
            nc.scalar.activation(out=wexp, in_=zwarm[:, 0:1],
                                 func=mybir.ActivationFunctionType.Exp)

            def ap_of(t, off, dims):
                return bass.AP(tensor=t.tensor, offset=t.offset + off,
                               ap=[list(t.ap[0])] + dims)

            # [WIN, D] const broadcast over windows: [WIN, nwin, D]
            def bc(t, nwin):
                return ap_of(t, 0, [[0, nwin], [1, D]])

            def bc_h(t, nwin):  # same, halved inner for the sin ops
                return ap_of(t, 0, [[0, nwin], [32, 2], [1, 32]])

            # rotate-half read of a [WIN, NW, D] row from window w0
            def rot_view(t, w0, nwin):
                return ap_of(t, w0 * D + 32, [[D, nwin], [-32, 2], [1, 32]])

            # plain window-segment view of a [WIN, n, D] tile
            def seg_view(t, s0, nwin, halved):
                inner = [[32, 2], [1, 32]] if halved else [[1, D]]
                return ap_of(t, s0 * D, [[D, nwin]] + inner)

            for r in range(RPC):
                q_row, k_row, v_row = row_tiles[r]
                out_row = rows.tile([WIN, NW, D], F32, tag="out_row")
                if r > 0:
                    nc.sync.dma_start(out=q_row, in_=q_d[r])
                    nc.sync.dma_start(out=k_row, in_=k_d[r])
                    nc.sync.dma_start(out=v_row, in_=v_d[r])
                # v with a ones column: makes PV emit the softmax denominator
                # as column D of a plain 65-wide matmul (no 1-col matmuls,
                # which trip walrus when combined with PSUM reciprocal).
                vo_row = rows.tile([WIN, NW, D + 1], BF16, tag="vo_row")
                nc.vector.tensor_copy(out=vo_row[:, :, 0:D], in_=v_row)
                nc.vector.memset(vo_row[:, :, D:D + 1], 1.0)

                # ---- rotary products.
                # q: both flavors (A, B) in one DVE op via stride-0 double
                # read; tiles hold [WIN, 2, nwin, D] (flavor-major).
                # Segments are issued interleaved with the half-loop (one
                # segment of lookahead) so row-start rotary bursts don't
                # head-of-line-block the DVE/Pool queues.
                SEG = 8
                qc = rot.tile([WIN, 2, NW + 1, D], BF16, tag="qc")
                qs = rot.tile([WIN, 2, NW + 1, D], BF16, tag="qs")
                kc = rot.tile([WIN, NW, D], BF16, tag="kc")
                ks = rot.tile([WIN, NW, D], BF16, tag="ks")

                def rot_seg(s0, n=SEG):
                    sl = slice(s0, s0 + n)
                    for f, (cn, sn) in enumerate((("cqA", "sqA"),
                                                  ("cqB", "sqB"))):
                        ceng = nc.vector if f == 0 else nc.gpsimd
                        ceng.tensor_mul(
                            seg_view(qc[:, f], s0, n, False),
                            q_row[:, sl, :], bc(c_sb[cn], n))
                        seng = nc.vector if f == 0 else nc.gpsimd
                        seng.tensor_mul(
                            seg_view(qs[:, f], s0, n, True),
                            rot_view(q_row, s0, n), bc_h(c_sb[sn], n))
                    nc.gpsimd.tensor_mul(
                        kc[:, sl, :], k_row[:, sl, :], bc(c_sb["cK"], n))
                    nc.gpsimd.tensor_mul(
                        ks[:, sl, :].rearrange("p w (h d2) -> p w h d2", h=2),
                        rot_view(k_row, s0, n),
                        bc_h(c_sb["sK"], n))
                    # 16-bit PSUM transpose-accumulation is broken on HW:
                    # pre-add cos+sin here and transpose once per flavor.
                    qcv = ap_of(qc, s0 * D, [[(NW + 1) * D, 2], [D, n], [1, D]])
                    qsv = ap_of(qs, s0 * D, [[(NW + 1) * D, 2], [D, n], [1, D]])
                    nc.vector.tensor_add(qcv, qcv, qsv)
                    nc.gpsimd.tensor_add(kc[:, sl, :], kc[:, sl, :],
                                         ks[:, sl, :])

                if r == 0:
                    rot_seg(0, 2)
                    rot_seg(2, 6)
                else:
                    rot_seg(0)
                rot_seg(SEG)

                # ---- main loop: 2 window-pairs (4 windows) per exp batch
                exp2_prev = None
                for half in range(NW // 4):
                    s_next = (half // 2 + 2) * SEG
                    if half % 2 == 0 and s_next < NW:
                        rot_seg(s_next)
                    if half == 0:
                        # zero-pad window NW for the qB flavor (the last
                        # pair reads c+2 == NW)
                        nc.vector.memset(qc[:, 1, NW, :], 0.0)
                    sim4 = psim_pool.tile([WIN, 8 * WIN], F32)
                    exp4 = exp_pool.tile([WIN, 8 * WIN], BF16, tag="exp4")
                    # both pairs' transposes into one PSUM bank:
                    # TB2[:, p, :] = [qA^T | qB^T | k^T] for pair p
                    TB2 = ptr_pool.tile([WIN, 2, 3 * WIN], BF16)
                    if r == 0 and half == 0:
                        # PE p-state warmup in the DMA-latency prologue; the
                        # real pair-0 transposes overwrite via start=True.
                        for i in range(28):
                            nc.tensor.matmul(
                                TB2[0:64, 0, 0:WIN], lhsT=zwarm, rhs=ident_sb,
                                is_transpose=True, start=True, stop=True)
                    for p in range(2):
                        c = 4 * half + 2 * p   # chunks (c, c+1)
                        for col, (ct, w0) in enumerate((
                            (qc[:, 0], c),
                            (qc[:, 1], c + 1),
                            (kc, c),
                        )):
                            sl = TB2[:, p, col * WIN: (col + 1) * WIN]
                            nc.tensor.matmul(
                                sl, lhsT=ct[:, w0: w0 + 2, :], rhs=ident_sb,
                                is_transpose=True, start=True, stop=True)

                    # one batched PSUM->SBUF copy for the half, then the
                    # odd-window operands shifted to partition base 0 on
                    # GPSIMD (base-64 matmul operands are broken on HW)
                    S2 = s_pool.tile([WIN, 2, 3 * WIN], BF16, tag="S2")
                    nc.vector.tensor_copy(S2, TB2)
                    S2o = s_pool.tile([64, 2, 3 * WIN], BF16, tag="S2o")
                    nc.gpsimd.tensor_copy(out=S2o, in_=S2[64:128, :, :])

                    for p in range(2):
                        # QK -> sim cols [own_c | prev_c+1 | own_c+1 | prev_c+2]
                        # own-chunk logits get the causal mask fused in as an
                        # accumulating matmul: Lmat^T @ (-30000*I) adds -30000
                        # wherever j > i, so exp underflows to exactly 0.
                        base = 4 * WIN * p
                        for par, S in enumerate((S2[:, p, :], S2o[:, p, :])):
                            ob = base + 2 * WIN * par
                            nc.tensor.matmul(
                                sim4[:, ob: ob + WIN],
                                lhsT=S[0:64, 2 * WIN: 3 * WIN],
                                rhs=S[0:64, 0: WIN],
                                start=True, stop=False)
                            nc.tensor.matmul(
                                sim4[:, ob: ob + WIN],
                                lhsT=mask_sb, rhs=negI_sb,
                                start=False, stop=True)
                            nc.tensor.matmul(
                                sim4[:, ob + WIN: ob + 2 * WIN],
                                lhsT=S[0:64, 2 * WIN: 3 * WIN],
                                rhs=S[0:64, WIN: 2 * WIN],
                                start=True, stop=True)

                    # ---- exp over 4 windows, then causal mask on own cols
                    nc.scalar.activation(
                        out=exp4, in_=sim4,
                        func=mybir.ActivationFunctionType.Exp)

                    # ---- PV + denominator + normalize, per pair
                    po4 = po_pool.tile([WIN, 4, D + 2], F32)
                    for p in range(2):
                        c = 4 * half + 2 * p
                        base = 4 * WIN * p
                        for j, w in enumerate((c, c + 1)):
                            osl = po4[:, 2 * p + j, 0:D + 1]
                            own = exp4[:, base + 2 * WIN * j:
                                       base + 2 * WIN * j + WIN]
                            if w == 0:
                                nc.tensor.matmul(osl, lhsT=own,
                                                 rhs=vo_row[:, w, :],
                                                 start=True, stop=True)
                            else:
                                if p == 0 and j == 0:
                                    prev = (exp2_prev[:, 7 * WIN: 8 * WIN]
                                            if exp2_prev is not None else None)
                                elif j == 0:
                                    prev = exp4[:, 3 * WIN: 4 * WIN]
                                else:
                                    prev = exp4[:, base + WIN: base + 2 * WIN]
                                nc.tensor.matmul(osl, lhsT=prev,
                                                 rhs=vo_row[:, w - 1, :],
                                                 start=True, stop=False)
                                nc.tensor.matmul(osl, lhsT=own,
                                                 rhs=vo_row[:, w, :],
                                                 start=False, stop=True)

                    # normalize: reciprocal of the 4 denominators, then
                    # per-window copy-with-scale (divide does not lower to HW)
                    # normalize: per-pair reciprocal of the denominators
                    # (GPSIMD cannot touch PSUM on HW, divide does not lower;
                    # DVE reciprocal + DVE/ACT per-window scale are legal)
                    rec4 = s_pool.tile([WIN, 4], F32, tag="rec4")
                    for p in range(2):
                        den_view = bass.AP(
                            tensor=po4.tensor,
                            offset=po4.offset + 2 * p * (D + 2) + D,
                            ap=[list(po4.ap[0]), [D + 2, 2]])
                        nc.vector.reciprocal(rec4[:, 2 * p: 2 * p + 2],
                                             den_view)
                    for w in range(4):
                        if w % 2 == 0:
                            nc.scalar.activation(
                                out=out_row[:, 4 * half + w, :],
                                in_=po4[:, w, 0:D],
                                func=mybir.ActivationFunctionType.Copy,
                                scale=rec4[:, w: w + 1])
                        else:
                            nc.vector.tensor_scalar_mul(
                                out_row[:, 4 * half + w, :],
                                po4[:, w, 0:D], rec4[:, w: w + 1])

                    exp2_prev = exp4
                    # ship finished output in 8-window pieces to overlap the
                    # final DMA with compute and shorten the tail
                    if r == RPC - 1:
                        w0 = 4 * half
                        nc.sync.dma_start(out=o_d[r][:, w0: w0 + 4, :],
                                          in_=out_row[:, w0: w0 + 4, :])
                    elif half % 2 == 1:
                        w0 = 4 * (half - 1)
                        nc.sync.dma_start(out=o_d[r][:, w0: w0 + 8, :],
                                          in_=out_row[:, w0: w0 + 8, :])

    nc.compile()
    return nc


_NC_CACHE = None


def _get_nc():
    global _NC_CACHE
    if _NC_CACHE is None:
        _NC_CACHE = build_bass()
    return _NC_CACHE


def _wmajor(a):
    # [ROWS, N, D] -> [ROWS, WIN, NW, D]: position-in-window major
    return np.ascontiguousarray(
        a.reshape(ROWS, NW, WIN, D).transpose(0, 2, 1, 3))


def _in_maps(q, k, v):
    q = _wmajor(np.asarray(q, dtype=np.float32).reshape(ROWS, N, D)).astype(BF)
    k = _wmajor(np.asarray(k, dtype=np.float32).reshape(ROWS, N, D)).astype(BF)
    v = _wmajor(np.asarray(v, dtype=np.float32).reshape(ROWS, N, D)).astype(BF)
    consts = _rot_consts()
    # mask in [j, i] orientation: keep i >= j
    maskT = np.triu(np.ones((WIN, WIN)), k=1).astype(BF)  # Lmat[p, j] = j > p
    maps = []
    for c in range(NCORES):
        sl = slice(c * RPC, (c + 1) * RPC)
        m = {
            "q": np.ascontiguousarray(q[sl]),
            "k": np.ascontiguousarray(k[sl]),
            "v": np.ascontiguousarray(v[sl]),
            "maskT": maskT,
        }
        for name, arr in consts.items():
            m[name] = arr.reshape(CONST_SHAPES[name])
        maps.append(m)
    return maps


def _run(q, k, v, **kw):
    nc = _get_nc()
    res = run_bass_kernel_spmd(nc, _in_maps(q, k, v), list(range(NCORES)), **kw)
    out = np.concatenate([res.results[c]["o"] for c in range(NCORES)], axis=0)
    out = out.transpose(0, 2, 1, 3).reshape(B, H, N, D)
    return np.ascontiguousarray(out), res


def kernel(q, k, v):
    out, _ = _run(q, k, v)
    return out


# revision 3
# speedup vs baseline: 1.0291x; 1.0291x over previous
"""Local (windowed) attention with rotary embeddings on 8 TRN2 NeuronCores. v2.

Same math as the fp32 baseline (window=128, look_backward=1, rotary via the
R-composition trick: k rotated once, q rotated twice with angles i and i+128),
rebuilt in bf16 around both the CoreSim cost model and the real-HW lowering
constraints discovered by bisection:
  - bf16 everywhere (tolerance 2e-2): matmul/transpose at 1 cyc/col vs fp32's
    4/2, DVE 2x on bf16 elementwise, DMA volume halved.
  - 16-bit PSUM transpose-ACCUMULATION is silently wrong on HW, so the
    rotary cos+sin add happens on DVE/GPSIMD and each operand is transposed
    exactly once (also halves the PE transpose column count).
  - Matmul operands at partition base 64 hang or corrupt on HW: the
    odd-window operands are shifted to base 0 by a batched GPSIMD copy
    (GPSIMD is the only engine that can move data across partitions).
  - GPSIMD cannot access PSUM on HW, and the divide ALU does not lower on
    any engine: normalize = DVE reciprocal of the PV ones-column denominators
    (PSUM-strided, per pair) + per-window copy-with-scale split between ACT
    and DVE tensor_scalar_mul.
  - Causal mask fused into QK as an accumulating matmul
    (strict-upper-triangle @ -30000*identity) so exp underflows to exactly 0
    -- no elementwise mask op at all.
  - exp on ACT batched over 4 windows [128, 1024]; transposes batched per
    2 window-pairs into one PSUM bank; identity built in-SBUF via
    affine_select; PE p-state warmed up with throwaway transposes during the
    DMA prologue; inputs/outputs DMA'd in pieces to shorten prologue/tail.
"""

import numpy as np
import ml_dtypes

import concourse.bass as bass
import concourse.bacc as bacc
import concourse.tile as tile
from concourse import mybir
from concourse.bass_utils import run_bass_kernel_spmd

B, H, N, D = 4, 8, 4096, 64
WIN = 128
NW = N // WIN            # 32 windows per row
NCORES = 8
ROWS = B * H             # 32 packed batch rows
RPC = ROWS // NCORES     # 4 rows per core
ROPE = 10000.0
SCALE = D ** -0.5

F32 = mybir.dt.float32
BF16 = mybir.dt.bfloat16
BF = ml_dtypes.bfloat16


def _rot_consts():
    """Host-side rotary constant tables, [WIN, D] each, bf16."""
    inv = 1.0 / (ROPE ** (np.arange(0, D, 2, dtype=np.float64) / D))  # [D/2]

    def mats(t):
        fr = t[:, None] * inv[None, :]
        fr = np.concatenate([fr, fr], axis=-1)  # [WIN, D]
        return np.cos(fr), np.sin(fr)

    i = np.arange(WIN, dtype=np.float64)
    cosA, sinA = mats(i)          # q angle i     (vs own chunk, k angle jj')
    cosB, sinB = mats(i + WIN)    # q angle i+128 (vs prev chunk)
    cosK, sinK = mats(i)          # k angle jj'

    def fold_sin(s):
        f = s.copy()
        f[:, : D // 2] = -f[:, : D // 2]
        return f

    out = dict(
        cqA=cosA * SCALE, sqA=fold_sin(sinA) * SCALE,
        cqB=cosB * SCALE, sqB=fold_sin(sinB) * SCALE,
        cK=cosK, sK=fold_sin(sinK),
    )
    return {k: v.astype(BF) for k, v in out.items()}


CONST_SHAPES = {n: [WIN, D] for n in ("cqA", "sqA", "cqB", "sqB", "cK", "sK")}


def build_bass():
    nc = bacc.Bacc("TRN2", target_bir_lowering=False)
    q_d = nc.declare_dram_parameter("q", [RPC, WIN, NW, D], BF16, isOutput=False)
    k_d = nc.declare_dram_parameter("k", [RPC, WIN, NW, D], BF16, isOutput=False)
    v_d = nc.declare_dram_parameter("v", [RPC, WIN, NW, D], BF16, isOutput=False)
    consts_d = {
        name: nc.declare_dram_parameter(name, shape, BF16, isOutput=False)
        for name, shape in CONST_SHAPES.items()
    }
    mask_d = nc.declare_dram_parameter("maskT", [WIN, WIN], BF16, isOutput=False)
    o_d = nc.declare_dram_parameter("o", [RPC, WIN, NW, D], F32, isOutput=True)

    with tile.TileContext(nc) as tc:
        with (
            tc.tile_pool(name="singles", bufs=1) as singles,
            tc.tile_pool(name="rows", bufs=3) as rows,
            tc.tile_pool(name="rot", bufs=2) as rot,
            tc.tile_pool(name="sS", bufs=4) as s_pool,
            tc.tile_pool(name="exp", bufs=3) as exp_pool,
            tc.tile_pool(name="ptr", bufs=2, space="PSUM") as ptr_pool,
            tc.tile_pool(name="psim", bufs=2, space="PSUM") as psim_pool,
            tc.tile_pool(name="po", bufs=2, space="PSUM") as po_pool,
        ):
            # ---- row 0 input DMAs first so rotary can start ASAP, then
            # constants in first-use order.
            row_tiles = []
            for r in range(RPC):
                q_row = rows.tile([WIN, NW, D], BF16, tag="q_row")
                k_row = rows.tile([WIN, NW, D], BF16, tag="k_row")
                v_row = rows.tile([WIN, NW, D], BF16, tag="v_row")
                row_tiles.append((q_row, k_row, v_row))
            # first 8 windows of row-0 q/k land first so rotary starts ASAP
            nc.sync.dma_start(out=row_tiles[0][0][:, 0:8, :],
                              in_=q_d[0][:, 0:8, :])
            nc.sync.dma_start(out=row_tiles[0][1][:, 0:8, :],
                              in_=k_d[0][:, 0:8, :])
            c_sb = {}
            for name, shape in CONST_SHAPES.items():
                t = singles.tile(shape, BF16, tag=f"const_{name}")
                nc.sync.dma_start(out=t, in_=consts_d[name][:, :])
                c_sb[name] = t
            nc.sync.dma_start(out=row_tiles[0][0][:, 8:, :],
                              in_=q_d[0][:, 8:, :])
            nc.sync.dma_start(out=row_tiles[0][1][:, 8:, :],
                              in_=k_d[0][:, 8:, :])
            nc.sync.dma_start(out=row_tiles[0][2], in_=v_d[0])
            mask_sb = singles.tile([WIN, WIN], BF16, tag="maskT")
            nc.sync.dma_start(out=mask_sb, in_=mask_d[:, :])
            ones_sb = singles.tile([WIN, 1], BF16, tag="ones")
            nc.vector.memset(ones_sb, 1.0)

            # ---- identity built in-SBUF (no DMA wait): ones where col == p
            ident_sb = singles.tile([WIN, WIN], BF16, tag="ident")
            ones_bc = bass.AP(tensor=ones_sb.tensor, offset=ones_sb.offset,
                              ap=[list(ones_sb.ap[0]), [0, WIN]])
            nc.gpsimd.affine_select(
                out=ident_sb, in_=ones_bc,
                compare_op=mybir.AluOpType.is_equal,
                fill=0.0, base=0, pattern=[[1, WIN]], channel_multiplier=-1)

            # -30000 * identity: rhs of the causal-mask matmul
            negI_sb = singles.tile([WIN, WIN], BF16, tag="negI")
            nc.vector.tensor_scalar_mul(negI_sb, ident_sb, -30000.0)

            # ---- PE p-state warmup: ~3us of throwaway transposes during the
            # DMA-latency prologue so real work starts at full clock; dummy
            # exp pulls the ACT table load off the critical path.
            zwarm = singles.tile([WIN, 64], BF16, tag="zwarm")
            nc.vector.memset(zwarm, 0.0)
            wexp = singles.tile([WIN, 1], BF16, tag="wexp")
            nc.scalar.activation(out=wexp, in_=zwarm[:, 0:1],
                                 func=mybir.ActivationFunctionType.Exp)

            def ap_of(t, off, dims):
                return bass.AP(tensor=t.tensor, offset=t.offset + off,
                               ap=[list(t.ap[0])] + dims)

            # [WIN, D] const broadcast over windows: [WIN, nwin, D]
            def bc(t, nwin):
                return ap_of(t, 0, [[0, nwin], [1, D]])

            def bc_h(t, nwin):  # same, halved inner for the sin ops
                return ap_of(t, 0, [[0, nwin], [32, 2], [1, 32]])

            # rotate-half read of a [WIN, NW, D] row from window w0
            def rot_view(t, w0, nwin):
                return ap_of(t, w0 * D + 32, [[D, nwin], [-32, 2], [1, 32]])

            # plain window-segment view of a [WIN, n, D] tile
            def seg_view(t, s0, nwin, halved):
                inner = [[32, 2], [1, 32]] if halved else [[1, D]]
                return ap_of(t, s0 * D, [[D, nwin]] + inner)

            for r in range(RPC):
                q_row, k_row, v_row = row_tiles[r]
                out_row = rows.tile([WIN, NW, D], F32, tag="out_row")
                if r > 0:
                    nc.sync.dma_start(out=q_row, in_=q_d[r])
                    nc.sync.dma_start(out=k_row, in_=k_d[r])
                    nc.sync.dma_start(out=v_row, in_=v_d[r])
                # v with a ones column: makes PV emit the softmax denominator
                # as column D of a plain 65-wide matmul (no 1-col matmuls,
                # which trip walrus when combined with PSUM reciprocal).
                vo_row = rows.tile([WIN, NW, D + 1], BF16, tag="vo_row")
                nc.vector.tensor_copy(out=vo_row[:, :, 0:D], in_=v_row)
                nc.vector.memset(vo_row[:, :, D:D + 1], 1.0)

                # ---- rotary products.
                # q: both flavors (A, B) in one DVE op via stride-0 double
                # read; tiles hold [WIN, 2, nwin, D] (flavor-major).
                # Segments are issued interleaved with the half-loop (one
                # segment of lookahead) so row-start rotary bursts don't
                # head-of-line-block the DVE/Pool queues.
                SEG = 8
                qc = rot.tile([WIN, 2, NW + 1, D], BF16, tag="qc")
                qs = rot.tile([WIN, 2, NW + 1, D], BF16, tag="qs")
                kc = rot.tile([WIN, NW, D], BF16, tag="kc")
                ks = rot.tile([WIN, NW, D], BF16, tag="ks")

                def rot_seg(s0, n=SEG):
                    sl = slice(s0, s0 + n)
                    for f, (cn, sn) in enumerate((("cqA", "sqA"),
                                                  ("cqB", "sqB"))):
                        ceng = nc.vector if f == 0 else nc.gpsimd
                        ceng.tensor_mul(
                            seg_view(qc[:, f], s0, n, False),
                            q_row[:, sl, :], bc(c_sb[cn], n))
                        seng = nc.vector if f == 0 else nc.gpsimd
                        seng.tensor_mul(
                            seg_view(qs[:, f], s0, n, True),
                            rot_view(q_row, s0, n), bc_h(c_sb[sn], n))
                    nc.gpsimd.tensor_mul(
                        kc[:, sl, :], k_row[:, sl, :], bc(c_sb["cK"], n))
                    nc.gpsimd.tensor_mul(
                        ks[:, sl, :].rearrange("p w (h d2) -> p w h d2", h=2),
                        rot_view(k_row, s0, n),
                        bc_h(c_sb["sK"], n))
                    # 16-bit PSUM transpose-accumulation is broken on HW:
                    # pre-add cos+sin here and transpose once per flavor.
                    qcv = ap_of(qc, s0 * D, [[(NW + 1) * D, 2], [D, n], [1, D]])
                    qsv = ap_of(qs, s0 * D, [[(NW + 1) * D, 2], [D, n], [1, D]])
                    nc.vector.tensor_add(qcv, qcv, qsv)
                    nc.gpsimd.tensor_add(kc[:, sl, :], kc[:, sl, :],
                                         ks[:, sl, :])

                if r == 0:
                    rot_seg(0, 2)
                    rot_seg(2, 6)
                else:
                    rot_seg(0)
                rot_seg(SEG)

                # ---- main loop: 2 window-pairs (4 windows) per exp batch
                exp2_prev = None
                for half in range(NW // 4):
                    s_next = (half // 2 + 2) * SEG
                    if half % 2 == 0 and s_next < NW:
                        rot_seg(s_next)
                    if half == 0:
                        # zero-pad window NW for the qB flavor (the last
                        # pair reads c+2 == NW)
                        nc.vector.memset(qc[:, 1, NW, :], 0.0)
                    sim4 = psim_pool.tile([WIN, 8 * WIN], F32)
                    exp4 = exp_pool.tile([WIN, 8 * WIN], BF16, tag="exp4")
                    # both pairs' transposes into one PSUM bank:
                    # TB2[:, p, :] = [qA^T | qB^T | k^T] for pair p
                    TB2 = ptr_pool.tile([WIN, 2, 3 * WIN], BF16)
                    if r == 0 and half == 0:
                        # PE p-state warmup in the DMA-latency prologue; the
                        # real pair-0 transposes overwrite via start=True.
                        for i in range(28):
                            nc.tensor.matmul(
                                TB2[0:64, 0, 0:WIN], lhsT=zwarm, rhs=ident_sb,
                                is_transpose=True, start=True, stop=True)
                    for p in range(2):
                        c = 4 * half + 2 * p   # chunks (c, c+1)
                        for col, (ct, w0) in enumerate((
                            (qc[:, 0], c),
                            (qc[:, 1], c + 1),
                            (kc, c),
                        )):
                            sl = TB2[:, p, col * WIN: (col + 1) * WIN]
                            nc.tensor.matmul(
                                sl, lhsT=ct[:, w0: w0 + 2, :], rhs=ident_sb,
                                is_transpose=True, start=True, stop=True)

                    # one batched PSUM->SBUF copy for the half, then the
                    # odd-window operands shifted to partition base 0 on
                    # GPSIMD (base-64 matmul operands are broken on HW)
                    S2 = s_pool.tile([WIN, 2, 3 * WIN], BF16, tag="S2")
                    nc.vector.tensor_copy(S2, TB2)
                    S2o = s_pool.tile([64, 2, 3 * WIN], BF16, tag="S2o")
                    nc.gpsimd.tensor_copy(out=S2o, in_=S2[64:128, :, :])

                    for p in range(2):
                        # QK -> sim cols [own_c | prev_c+1 | own_c+1 | prev_c+2]
                        # own-chunk logits get the causal mask fused in as an
                        # accumulating matmul: Lmat^T @ (-30000*I) adds -30000
                        # wherever j > i, so exp underflows to exactly 0.
                        base = 4 * WIN * p
                        for par, S in enumerate((S2[:, p, :], S2o[:, p, :])):
                            ob = base + 2 * WIN * par
                            nc.tensor.matmul(
                                sim4[:, ob: ob + WIN],
                                lhsT=S[0:64, 2 * WIN: 3 * WIN],
                                rhs=S[0:64, 0: WIN],
                                start=True, stop=False)
                            nc.tensor.matmul(
                                sim4[:, ob: ob + WIN],
                                lhsT=mask_sb, rhs=negI_sb,
                                start=False, stop=True)
                            nc.tensor.matmul(
                                sim4[:, ob + WIN: ob + 2 * WIN],
                                lhsT=S[0:64, 2 * WIN: 3 * WIN],
                                rhs=S[0:64, WIN: 2 * WIN],
                                start=True, stop=True)

                    # ---- exp over 4 windows, then causal mask on own cols
                    nc.scalar.activation(
                        out=exp4, in_=sim4,
                        func=mybir.ActivationFunctionType.Exp)

                    # ---- PV + denominator + normalize, per pair
                    po4 = po_pool.tile([WIN, 4, D + 2], F32)
                    for p in range(2):
                        c = 4 * half + 2 * p
                        base = 4 * WIN * p
                        for j, w in enumerate((c, c + 1)):
                            osl = po4[:, 2 * p + j, 0:D + 1]
                            own = exp4[:, base + 2 * WIN * j:
                                       base + 2 * WIN * j + WIN]
                            if w == 0:
                                nc.tensor.matmul(osl, lhsT=own,
                                                 rhs=vo_row[:, w, :],
                                                 start=True, stop=True)
                            else:
                                if p == 0 and j == 0:
                                    prev = (exp2_prev[:, 7 * WIN: 8 * WIN]
                                            if exp2_prev is not None else None)
                                elif j == 0:
                                    prev = exp4[:, 3 * WIN: 4 * WIN]
                                else:
                                    prev = exp4[:, base + WIN: base + 2 * WIN]
                                nc.tensor.matmul(osl, lhsT=prev,
                                                 rhs=vo_row[:, w - 1, :],
                                                 start=True, stop=False)
                                nc.tensor.matmul(osl, lhsT=own,
                                                 rhs=vo_row[:, w, :],
                                                 start=False, stop=True)

                    # normalize: reciprocal of the 4 denominators, then
                    # per-window copy-with-scale (divide does not lower to HW)
                    # normalize: per-pair reciprocal of the denominators
                    # (GPSIMD cannot touch PSUM on HW, divide does not lower;
                    # DVE reciprocal + DVE/ACT per-window scale are legal)
                    rec4 = s_pool.tile([WIN, 4], F32, tag="rec4")
                    for p in range(2):
                        den_view = bass.AP(
                            tensor=po4.tensor,
                            offset=po4.offset + 2 * p * (D + 2) + D,
                            ap=[list(po4.ap[0]), [D + 2, 2]])
                        nc.vector.reciprocal(rec4[:, 2 * p: 2 * p + 2],
                                             den_view)
                    for w in range(4):
                        if w % 2 == 0:
                            nc.scalar.activation(
                                out=out_row[:, 4 * half + w, :],
                                in_=po4[:, w, 0:D],
                                func=mybir.ActivationFunctionType.Copy,
                                scale=rec4[:, w: w + 1])
                        else:
                            nc.vector.tensor_scalar_mul(
                                out_row[:, 4 * half + w, :],
                                po4[:, w, 0:D], rec4[:, w: w + 1])

                    exp2_prev = exp4
                    # ship finished output in 8-window pieces to overlap the
                    # final DMA with compute and shorten the tail
                    if r == RPC - 1:
                        w0 = 4 * half
                        nc.sync.dma_start(out=o_d[r][:, w0: w0 + 4, :],
                                          in_=out_row[:, w0: w0 + 4, :])
                    elif half % 2 == 1:
                        w0 = 4 * (half - 1)
                        nc.sync.dma_start(out=o_d[r][:, w0: w0 + 8, :],
                                          in_=out_row[:, w0: w0 + 8, :])

    nc.compile()
    return nc


_NC_CACHE = None


def _get_nc():
    global _NC_CACHE
    if _NC_CACHE is None:
        _NC_CACHE = build_bass()
    return _NC_CACHE


def _wmajor(a):
    # [ROWS, N, D] -> [ROWS, WIN, NW, D]: position-in-window major
    return np.ascontiguousarray(
        a.reshape(ROWS, NW, WIN, D).transpose(0, 2, 1, 3))


def _in_maps(q, k, v):
    q = _wmajor(np.asarray(q, dtype=np.float32).reshape(ROWS, N, D)).astype(BF)
    k = _wmajor(np.asarray(k, dtype=np.float32).reshape(ROWS, N, D)).astype(BF)
    v = _wmajor(np.asarray(v, dtype=np.float32).reshape(ROWS, N, D)).astype(BF)
    consts = _rot_consts()
    # mask in [j, i] orientation: keep i >= j
    maskT = np.triu(np.ones((WIN, WIN)), k=1).astype(BF)  # Lmat[p, j] = j > p
    maps = []
    for c in range(NCORES):
        sl = slice(c * RPC, (c + 1) * RPC)
        m = {
            "q": np.ascontiguousarray(q[sl]),
            "k": np.ascontiguousarray(k[sl]),
            "v": np.ascontiguousarray(v[sl]),
            "maskT": maskT,
        }
        for name, arr in consts.items():
            m[name] = arr.reshape(CONST_SHAPES[name])
        maps.append(m)
    return maps


def _run(q, k, v, **kw):
    nc = _get_nc()
    res = run_bass_kernel_spmd(nc, _in_maps(q, k, v), list(range(NCORES)), **kw)
    out = np.concatenate([res.results[c]["o"] for c in range(NCORES)], axis=0)
    out = out.transpose(0, 2, 1, 3).reshape(B, H, N, D)
    return np.ascontiguousarray(out), res


def kernel(q, k, v):
    out, _ = _run(q, k, v)
    return out
